# revision 9
# baseline (speedup 1.0000x reference)
"""Trainium2 Bass kernel for nn_Attention_62672162783289.

Dense transformer attention block: LayerNorm -> fused QKV -> per-head scaled
dot-product attention with gathered relative-position bias -> output proj.

Sharding: data-parallel over batch B=16 across 8 NeuronCores (2 batches/core).
No collectives needed; outputs are concatenated on the host.

Device-side design (per core, all matmuls bf16 with fp32 PSUM accumulation):
  - The kernel is PE-bound: QKV (221k col-cycles) + scores/AV (393k) + proj
    (74k) is the irreducible bf16 matmul work. Everything else is arranged to
    stay off the PE and below the ScalarE exp stream (which sets the phase-2
    step period at ~16.6us: 16 x [128,1024] exp activations per head-pair).
  - LayerNorm stats run on VectorE (bn_stats/bn_aggr over a second token-major
    copy of x), not as ones-matmuls: PE does no stats work, and ScalarE does
    no Square pass. alpha/beta rows bounce through DRAM for the partition
    broadcast used by the xhat multiply (DVE).
  - rstd = exp(-0.5*ln(var+eps)) on ScalarE keeps every activation in the
    natural_log_exp_and_others table set: zero ACT table swaps in the kernel.
  - Q,K are produced feature-major in head-pair tiles (q_2p|q_2p+1 /
    k_2p|k_2p+1 on partition halves); scores are computed transposed S^T[m,n]
    so softmax normalization is a column sum obtained for free from an extra
    ones-column in the AV lhsT.
  - The position bias is applied per (mt,nb) tile according to PLAN: 'E'
    multiplies a precomputed exp(bias) into the exp'd scores on VectorE; 'I'
    adds the pre-scaled bias with identity matmuls on the PE. The mix
    balances DVE/PE load under the ScalarE exp period. (PSUM seeding via
    GpSimd or DMA is rejected by walrus — only PE/ACT/DVE reach PSUM.)
  - V is produced token-major [m, (h,65)] with an interleaved ones column so it
    can be used directly as the AV stationary operand; the V bias commutes
    through the softmax average and is folded into the proj bias on the host.
  - AV output arrives transposed [(d|ones), n]; after dividing by the ones-row
    (denominator, bcast via DRAM bounce), it is exactly the proj contraction
    layout.
"""
import os
import numpy as np
import ml_dtypes

import concourse.bass as bass
import concourse.tile as tile
from concourse import bacc, mybir

bf16 = mybir.dt.bfloat16
f32 = mybir.dt.float32
FP = mybir.ActivationFunctionType
ALU = mybir.AluOpType

B, RES, DIM, H, KD = 16, 32, 768, 12, 64
N = RES * RES            # 1024 tokens
DH = KD * H              # 768
NCORES = 8
BL = B // NCORES         # 2 batches per core
SCALE = KD ** -0.5
LN_EPS = 1e-5
CHUNKS = DIM // 128      # 6 contraction chunks
PAIRS = H // 2           # 6 head pairs
MT = N // 128            # 8 m-tiles
TT = N // 128            # 8 token tiles

# Bias application plan per (mt, nb) tile (k = 2*mt+nb), repeated each step.
# 'E' = exp(bias) multiply on DVE, 'I' = PE ident matmul. (PSUM seeding via
# GpSimd/DMA is rejected by walrus: only PE/ACT/DVE can touch PSUM.)
PLAN = os.environ.get("BIAS_PLAN", "EEEEEIEEEEIEEEEI")
assert len(PLAN) == 16 and set(PLAN) <= set("EI")


def _bcast_ap(dram_ap, nparts):
    """Partition-step-0 broadcast AP over a DRAM row region."""
    return bass.AP(tensor=dram_ap.tensor, offset=dram_ap.offset,
                   ap=[[0, nparts]] + dram_ap.ap[1:])


def build_program(reps=1):
    nc = bacc.Bacc("TRN2", target_bir_lowering=False, debug=False,
                   num_devices=NCORES)

    x_d = nc.dram_tensor("x", [BL, DIM, N], bf16, kind="ExternalInput").ap()
    xt_d = nc.dram_tensor("xt", [BL, TT, 128, DIM], bf16,
                          kind="ExternalInput").ap()
    wqk_d = nc.dram_tensor("wqk", [DIM, 2 * DH], bf16, kind="ExternalInput").ap()
    qkb_d = nc.dram_tensor("qkb", [H, 128, 1], f32, kind="ExternalInput").ap()
    wv_d = nc.dram_tensor("wv", [DIM, DH], bf16, kind="ExternalInput").ap()
    pw_d = nc.dram_tensor("pw", [DH, DIM], bf16, kind="ExternalInput").ap()
    pb_d = nc.dram_tensor("pb", [1, DIM], f32, kind="ExternalInput").ap()
    eb_d = nc.dram_tensor("eb", [H, MT, 128, N], bf16, kind="ExternalInput").ap()
    sb_d = nc.dram_tensor("sb", [H, MT, 128, N], bf16, kind="ExternalInput").ap()
    id_d = nc.dram_tensor("ident", [128, 128], bf16, kind="ExternalInput").ap()
    out_d = nc.dram_tensor("out", [BL, N, DIM], f32, kind="ExternalOutput").ap()

    den_scr = nc.dram_tensor("den_scr", [PAIRS * BL, 4 * 512], f32).ap()
    ab_scr = nc.dram_tensor("ab_scr", [BL, N], bf16).ap()
    bb_scr = nc.dram_tensor("bb_scr", [BL, N], bf16).ap()

    with tile.TileContext(nc) as tc:
        with (
            tc.tile_pool(name="persist", bufs=1) as persist,
            tc.tile_pool(name="qkvout", bufs=1) as qkvout,
        ):
            # ---- persistent weights / constants
            pw_sb = []
            for p in range(PAIRS):
                t = persist.tile([128, DIM], bf16, tag=f"pw{p}")
                nc.scalar.dma_start(t[:], pw_d[p * 128:(p + 1) * 128, :])
                pw_sb.append(t)
            projbB = persist.tile([128, DIM], f32, tag="projbB")
            nc.gpsimd.dma_start(projbB[:], _bcast_ap(pb_d[0:1, :], 128))
            ident = persist.tile([128, 128], bf16, tag="ident")
            nc.scalar.dma_start(ident[:], id_d[:])

            for rep in range(reps):
                _emit_body(nc, tc, persist, qkvout, rep,
                           x_d, xt_d, wqk_d, qkb_d, wv_d, eb_d, sb_d,
                           out_d, den_scr, ab_scr, bb_scr, pw_sb, projbB,
                           ident)

    nc.compile()
    return nc


def _emit_body(nc, tc, persist, qkvout, rep,
               x_d, xt_d, wqk_d, qkb_d, wv_d, eb_d, sb_d, out_d,
               den_scr, ab_scr, bb_scr, pw_sb, projbB, ident):
    # tags are shared across reps: WAR deps serialize reps (correct; for timing)
    r = ""
    qk_sb = [[None] * H for _ in range(BL)]  # 12 f-tiles: even=QPAIR, odd=KPAIR
    v_sb = [[None] * TT for _ in range(BL)]  # [b][tt] -> [128, H*65] bf16
    projIn = [[None] * PAIRS for _ in range(BL)]

    # ======== Phase 1: LayerNorm + QKV ========
    # Stats on DVE (bn_stats over token-major x), QKV on PE. Loads are spread
    # over the sync/scalar/vector/gpsimd DMA queues so x, xt and wqk stream in
    # parallel and the first QKV matmul can start ~8us in.
    with (
        tc.tile_pool(name="wqkp", bufs=1) as wqkp,
        tc.tile_pool(name="xp", bufs=12) as xp,
        tc.tile_pool(name="xtp", bufs=8) as xtp,
        tc.tile_pool(name="bnp", bufs=2) as bnp,
        tc.tile_pool(name="rowp", bufs=2) as rowp,
        tc.tile_pool(name="lnbc", bufs=2) as lnbc,
        tc.tile_pool(name="xhp", bufs=12) as xhp,
        tc.tile_pool(name="tmpp", bufs=2) as tmpp,
        tc.tile_pool(name="qkps", bufs=2, space="PSUM") as qkps,
        tc.tile_pool(name="vps", bufs=2, space="PSUM") as vps,
    ):
        # token-major x for stats: gpsimd queue (idle at start)
        xtc = [[None] * TT for _ in range(BL)]
        for b in range(BL):
            for t in range(TT):
                tl = xtp.tile([128, DIM], bf16, name="xtc", tag="xtc")
                nc.gpsimd.dma_start(tl[:], xt_d[b, t])
                xtc[b][t] = tl
        # feature-major x for xhat/QKV: sync queue
        xc = [[None] * CHUNKS for _ in range(BL)]
        for b in range(BL):
            for c in range(CHUNKS):
                t = xp.tile([128, N], bf16, name="xc", tag="xc")
                nc.sync.dma_start(t[:], x_d[b, c * 128:(c + 1) * 128, :])
                xc[b][c] = t
        # weights: scalar queue
        wqk_sb, wv_sb, qkb_sb = [], [], []
        for c in range(CHUNKS):
            t = wqkp.tile([128, 2 * DH], bf16, tag=f"wqk{c}")
            nc.scalar.dma_start(t[:], wqk_d[c * 128:(c + 1) * 128, :])
            wqk_sb.append(t)
        for c in range(CHUNKS):
            t = wqkp.tile([128, DH], bf16, tag=f"wv{c}")
            nc.scalar.dma_start(t[:], wv_d[c * 128:(c + 1) * 128, :])
            wv_sb.append(t)
        for ft in range(H):
            t = wqkp.tile([128, 1], f32, tag=f"qkb{ft}")
            nc.scalar.dma_start(t[:], qkb_d[ft])
            qkb_sb.append(t)
        eps_t = wqkp.tile([128, 1], f32, tag="eps")
        nc.vector.memset(eps_t[:], LN_EPS)

        bcasts = {}

        def stage_stats(b):
            """Per-token mean/var via bn_stats on token-major x, then
            alpha = rsqrt(var+eps), beta = -mu*alpha, DRAM bounce + bcast."""
            st8 = rowp.tile([128, TT, 2], f32, name="st8", tag="st8")
            for t in range(TT):
                bn6 = bnp.tile([128, 2, 6], f32, name="bn6", tag="bn6")
                nc.vector.bn_stats(bn6[:, 0, :], xtc[b][t][:, 0:384])
                nc.vector.bn_stats(bn6[:, 1, :], xtc[b][t][:, 384:768])
                nc.vector.bn_aggr(st8[:, t, :], bn6[:])
            lnv = rowp.tile([128, TT], f32, name="lnv", tag="lnv")
            nc.scalar.activation(lnv[:], st8[:, :, 1], FP.Ln, bias=eps_t[:])
            alpha8 = rowp.tile([128, TT], bf16, name="alpha8", tag="alpha8")
            nc.scalar.activation(alpha8[:], lnv[:], FP.Exp, scale=-0.5)
            beta8 = rowp.tile([128, TT], bf16, name="beta8", tag="beta8")
            nc.vector.scalar_tensor_tensor(beta8[:], st8[:, :, 0], -1.0,
                                           alpha8[:], ALU.mult, ALU.mult)
            # DRAM bounce: token t*128+p <- sbuf[p, t]
            a_dst = bass.AP(tensor=ab_scr.tensor, offset=ab_scr.offset + b * N,
                            ap=[[1, 128], [128, TT]])
            b_dst = bass.AP(tensor=bb_scr.tensor, offset=bb_scr.offset + b * N,
                            ap=[[1, 128], [128, TT]])
            nc.sync.dma_start(a_dst, alpha8[:])
            nc.sync.dma_start(b_dst, beta8[:])
            for tb in range(2):
                ts = slice(tb * 512, (tb + 1) * 512)
                alphaB = lnbc.tile([128, 512], bf16, tag="alphaB")
                nc.gpsimd.dma_start(alphaB[:],
                                    _bcast_ap(ab_scr[b:b + 1, ts], 128))
                betaB = lnbc.tile([128, 512], bf16, tag="betaB")
                nc.gpsimd.dma_start(betaB[:],
                                    _bcast_ap(bb_scr[b:b + 1, ts], 128))
                bcasts[(b, tb)] = (alphaB, betaB)

        xh = [[None] * CHUNKS for _ in range(BL)]

        def stage_c(b, tb):  # xhat = x*alpha + beta (bf16), one t-half
            alphaB, betaB = bcasts[(b, tb)]
            ts = slice(tb * 512, (tb + 1) * 512)
            for c in range(CHUNKS):
                if xh[b][c] is None:
                    xh[b][c] = xhp.tile([128, N], bf16, name="xh", tag="xh")
                t0 = tmpp.tile([128, 512], bf16, name="t0", tag="t0")
                nc.vector.tensor_mul(t0[:], xc[b][c][:, ts], alphaB[:])
                nc.vector.tensor_add(xh[b][c][:, ts], t0[:], betaB[:])

        def stage_d_qk(b, tb):  # QK matmuls for one t-half
            for ft in range(H):
                if qk_sb[b][ft] is None:
                    qk_sb[b][ft] = qkvout.tile([128, N], bf16,
                                               tag=f"{r}qk{b}_{ft}", name="qt")
                qt = qk_sb[b][ft]
                ps = qkps.tile([128, 512], f32, name="qkp", tag="qkp")
                for c in range(CHUNKS):
                    nc.tensor.matmul(
                        ps[:], wqk_sb[c][:, ft * 128:(ft + 1) * 128],
                        xh[b][c][:, tb * 512:(tb + 1) * 512],
                        start=(c == 0), stop=(c == CHUNKS - 1))
                nc.scalar.activation(qt[:, tb * 512:(tb + 1) * 512], ps[:],
                                     FP.Identity, bias=qkb_sb[ft][:])

        def stage_d_v(b, tts):  # V matmuls for the given token tiles
            for tt in tts:
                ps = vps.tile([128, DH], f32, name="vp", tag="vp")
                for c in range(CHUNKS):
                    lhs = xh[b][c][:, tt * 128:(tt + 1) * 128]
                    nc.tensor.matmul(ps[:, 0:512], lhs, wv_sb[c][:, 0:512],
                                     start=(c == 0), stop=(c == CHUNKS - 1))
                    nc.tensor.matmul(ps[:, 512:DH], lhs, wv_sb[c][:, 512:DH],
                                     start=(c == 0), stop=(c == CHUNKS - 1))
                # V bias is folded into the proj bias on the host (the AV
                # normalize makes it a constant per-dh additive term), so the
                # evacuation is a plain copy: on ScalarE, which idles in
                # phase 1, keeping VectorE free for stats/xhat.
                vt = qkvout.tile([128, H * 65], bf16, tag=f"{r}v{b}_{tt}")
                v_sb[b][tt] = vt
                vv = vt[:].rearrange("p (h d) -> p h d", d=65)
                nc.scalar.activation(
                    vv[:, :, 0:64],
                    ps[:].rearrange("p (h d) -> p h d", d=64), FP.Identity)
                nc.vector.memset(vv[:, :, 64:65], 1.0)

        stage_stats(0)
        stage_c(0, 0)
        stage_c(0, 1)
        stage_d_qk(0, 0)
        stage_stats(1)               # DVE prepares b1 while PE runs QKV(b0)
        stage_d_v(0, range(0, 4))
        stage_d_qk(0, 1); stage_d_v(0, range(4, TT))
        stage_c(1, 0); stage_c(1, 1)
        stage_d_qk(1, 0); stage_d_v(1, range(0, 4))
        stage_d_qk(1, 1); stage_d_v(1, range(4, TT))

    # ======== Phase 2: attention + projection, software-pipelined ========
    # Step i emits: scores+exp(+bias per PLAN) of i with the AV matmuls of i-1
    # interleaved per m-chunk, the deferred reciprocal+normalize of i-2
    # mid-step, and the AV-accumulator evacuation of i-1. proj(b0) is emitted
    # before the pipeline drain so the tensor engine has dense work while the
    # last step's exps stream through ScalarE.
    with (
        tc.tile_pool(name="ebp", bufs=20) as ebp,
        tc.tile_pool(name="ep", bufs=20) as ep,
        tc.tile_pool(name="denrp", bufs=1) as denrp,
        tc.tile_pool(name="recp", bufs=3) as recp,
        tc.tile_pool(name="tmpb", bufs=2) as tmpb,
        tc.tile_pool(name="outp", bufs=3) as outp,
        tc.tile_pool(name="sps", bufs=2, space="PSUM") as spsp,
        tc.tile_pool(name="avps", bufs=2, space="PSUM") as avpsp,
    ):
        steps = [(p, b) for p in range(PAIRS) for b in range(BL)]
        eb_cache = {}

        def load_bias(p):
            # tile k = (mt, nb): both heads' [128, 512] n-block side by side.
            # 'E' tiles hold exp(bias) (multiplied on VectorE); 'S'/'I' tiles
            # hold the pre-scaled additive bias.
            eb_sb = [None] * (2 * MT)
            for mt in range(MT):
                for nb in range(2):
                    k = 2 * mt + nb
                    t = ebp.tile([128, N], bf16, name="ebt", tag="ebt")
                    src_d = eb_d if PLAN[k] == 'E' else sb_d
                    base = src_d[2 * p, mt]
                    src = bass.AP(tensor=base.tensor,
                                  offset=base.offset + nb * 512,
                                  ap=[base.ap[0], [MT * 128 * N, 2], [1, 512]])
                    eng = nc.gpsimd if PLAN[k] == 'E' else nc.sync
                    eng.dma_start(
                        t[:].rearrange("p (h n) -> p h n", h=2), src)
                    eb_sb[k] = t
            return eb_sb

        def av_chunk(st, mt):
            p, b, E, avt = st["p"], st["b"], st["E"], st["avt"]
            for hh in range(2):
                h = 2 * p + hh
                lhsT = v_sb[b][mt][:, h * 65:(h + 1) * 65]
                for nb in range(2):
                    nc.tensor.matmul(
                        avt[hh][:, nb * 512:(nb + 1) * 512], lhsT,
                        E[2 * mt + nb][:, hh * 512:(hh + 1) * 512],
                        start=(mt == 0), stop=(mt == MT - 1))

        def evac_step(st):
            """Denominators out + unnormalized PSUM->SBUF evacuation, with the
            copies split across ScalarE/VectorE to balance their step load."""
            p, b, avt = st["p"], st["b"], st["avt"]
            denrow = denrp.tile([65, 4 * 512], f32, name="denrow", tag="denrow")
            # avt[hh] row 64 is [nb0|nb1] contiguous == den_scr quadrant order
            nc.vector.tensor_copy(denrow[64:65, 0:N], avt[0][64:65, :])
            nc.vector.tensor_copy(denrow[64:65, N:2 * N], avt[1][64:65, :])
            PI = qkvout.tile([128, N], bf16, tag=f"{r}qk{b}_{2 * p}")
            projIn[b][p] = PI
            tb_t = tmpb.tile([64, N], bf16, name="tbt", tag="tbt")
            nc.vector.tensor_copy(PI[0:64, :], avt[0][0:64, :])
            nc.vector.tensor_copy(tb_t[:], avt[1][0:64, :])
            nc.sync.dma_start(PI[64:128, :], tb_t[:])
            idx = p * BL + b
            nc.sync.dma_start(den_scr[idx:idx + 1, :], denrow[64:65, :])
            denB = recp.tile([128, N], f32, tag="recp", name="denB")
            for hh in range(2):
                for nb in range(2):
                    q = hh * 2 + nb
                    src = den_scr[idx:idx + 1, q * 512:(q + 1) * 512]
                    nc.gpsimd.dma_start(
                        denB[hh * 64:(hh + 1) * 64, nb * 512:(nb + 1) * 512],
                        _bcast_ap(src, 64))
            st["denB"] = denB
            st["PI"] = PI

        def norm_step(st):
            rB = recp.tile([128, N], f32, tag="recp", name="rB")
            nc.vector.reciprocal_approx_fast(rB[:], st["denB"][:])
            nc.vector.tensor_mul(st["PI"][:], st["PI"][:], rB[:])

        def emit_proj(b):
            for tt in range(TT):
                ps = spsp.tile([128, N], f32, name="sps", tag="sps")
                for p in range(PAIRS):
                    lhsT = projIn[b][p][:, tt * 128:(tt + 1) * 128]
                    nc.tensor.matmul(ps[:, 0:512], lhsT, pw_sb[p][:, 0:512],
                                     start=(p == 0), stop=(p == PAIRS - 1))
                    nc.tensor.matmul(ps[:, 512:DIM], lhsT, pw_sb[p][:, 512:DIM],
                                     start=(p == 0), stop=(p == PAIRS - 1))
                ot = outp.tile([128, DIM], f32, name="ot", tag="ot")
                nc.vector.tensor_add(ot[:], ps[:, 0:DIM], projbB[:])
                nc.scalar.dma_start(out_d[b, tt * 128:(tt + 1) * 128, :], ot[:])

        prev = None
        pend = None
        for p, b in steps:
            if b == 0:
                eb_cache[p] = load_bias(p)
            eb_sb = eb_cache[p]
            QP = qk_sb[b][2 * p]
            KP = qk_sb[b][2 * p + 1]
            E = [None] * (2 * MT)
            cur = {"p": p, "b": b, "E": E,
                   "avt": [avpsp.tile([65, N], f32, name="avt", tag="avt")
                           for _ in range(2)]}
            for mt in range(MT):
                # half-wide score tiles (both heads, one n-block) double-
                # buffered: the next tile's matmuls only wait on the exp two
                # tiles back, breaking the PE<->ScalarE alternation chain
                for nb in range(2):
                    k = 2 * mt + nb
                    v = PLAN[k]
                    sps = spsp.tile([128, N], f32, name="sps", tag="sps")
                    bt = eb_sb[k]
                    for hh in range(2):
                        sl = slice(hh * 64, (hh + 1) * 64)
                        lhsT = KP[sl, mt * 128:(mt + 1) * 128]
                        nc.tensor.matmul(
                            sps[:, hh * 512:(hh + 1) * 512], lhsT,
                            QP[sl, nb * 512:(nb + 1) * 512],
                            start=True, stop=(v != 'I'))
                    if v == 'I':
                        for hh in range(2):
                            nc.tensor.matmul(
                                sps[:, hh * 512:(hh + 1) * 512], ident[:],
                                bt[:, hh * 512:(hh + 1) * 512],
                                start=False, stop=True)
                    et = ep.tile([128, N], bf16, name="et", tag="et")
                    nc.scalar.activation(et[:], sps[:], FP.Exp, scale=SCALE)
                    if v == 'E':
                        nc.vector.tensor_mul(et[:], et[:], bt[:])
                    E[k] = et
                if prev is not None:
                    av_chunk(prev, mt)
                if mt == 3 and pend is not None:
                    norm_step(pend)
                    pend = None
            if prev is not None:
                evac_step(prev)
                pend = prev
            prev = cur
        # normalize the second-to-last step, then emit proj(b0): it fills the
        # PE while the final step's exps drain through ScalarE
        if pend is not None:
            norm_step(pend)
            pend = None
        emit_proj(0)
        # drain the pipeline
        for mt in range(MT):
            av_chunk(prev, mt)
        evac_step(prev)
        norm_step(prev)
        emit_proj(1)


# ---------------- host side ----------------

def _prep_inputs(x, ln_w, ln_b, qkv_w, qkv_b, proj_w, proj_b,
                 attn_biases, bias_idxs):
    """Fold LN affine into QKV weights; build device layouts (shared part)."""
    f64 = np.float64
    Wp = qkv_w.astype(f64) * ln_w.astype(f64)[None, :]       # [2304, 768]
    bp = qkv_b.astype(f64) + qkv_w.astype(f64) @ ln_b.astype(f64)

    def q_rows(h): return np.arange(h * 3 * KD, h * 3 * KD + KD)
    def k_rows(h): return np.arange(h * 3 * KD + KD, h * 3 * KD + 2 * KD)
    def v_rows(h): return np.arange(h * 3 * KD + 2 * KD, h * 3 * KD + 3 * KD)

    qk_order = []
    for p in range(PAIRS):
        qk_order += list(q_rows(2 * p)) + list(q_rows(2 * p + 1))
        qk_order += list(k_rows(2 * p)) + list(k_rows(2 * p + 1))
    qk_order = np.array(qk_order)
    v_order = np.concatenate([v_rows(h) for h in range(H)])

    wqk = np.ascontiguousarray(Wp[qk_order].T).astype(ml_dtypes.bfloat16)
    qkb = bp[qk_order].astype(np.float32).reshape(H, 128, 1)
    wv = np.ascontiguousarray(Wp[v_order].T).astype(ml_dtypes.bfloat16)
    pw = np.ascontiguousarray(proj_w.T).astype(ml_dtypes.bfloat16)
    # V bias (LN-folded qkv bias for the v rows) commutes through the
    # softmax average, so it folds exactly into the proj bias.
    bv = bp[v_order]                                         # [DH] natural dh
    pb = (proj_b.astype(f64) + proj_w.astype(f64) @ bv).astype(
        np.float32).reshape(1, DIM)

    biasT = attn_biases.astype(f64)[:, np.asarray(bias_idxs)].transpose(0, 2, 1)
    eb = np.ascontiguousarray(np.exp(biasT)).astype(
        ml_dtypes.bfloat16).reshape(H, MT, 128, N)
    sb = np.ascontiguousarray(biasT / SCALE).astype(
        ml_dtypes.bfloat16).reshape(H, MT, 128, N)
    ident = np.eye(128, dtype=ml_dtypes.bfloat16)
    return dict(wqk=wqk, qkb=qkb, wv=wv, pw=pw, pb=pb, eb=eb, sb=sb,
                ident=ident)


def _make_in_maps(x, shared):
    xr = x.reshape(NCORES, BL, N, DIM)
    xft = np.ascontiguousarray(xr.transpose(0, 1, 3, 2)).astype(
        ml_dtypes.bfloat16)
    xtk = np.ascontiguousarray(xr.reshape(NCORES, BL, TT, 128, DIM)).astype(
        ml_dtypes.bfloat16)
    return [dict(x=xft[i], xt=xtk[i], **shared) for i in range(NCORES)]


_PROG = {}


def _get_program(reps=1):
    if reps not in _PROG:
        _PROG[reps] = build_program(reps)
    return _PROG[reps]


class _Runner:
    """Persistent jitted SPMD executor (mirrors bass2jax.run_bass_via_pjrt's
    multi-core branch, but the jitted callable is cached across calls)."""

    def __init__(self, nc):
        import jax
        from jax.experimental.shard_map import shard_map
        from jax.sharding import Mesh, PartitionSpec
        from concourse import mybir as _mb
        from concourse.bass2jax import _bass_exec_p, install_neuronx_cc_hook

        install_neuronx_cc_hook()
        self.jax = jax
        from concourse.bass2jax import partition_id_tensor
        part_name = (nc.partition_id_tensor.name
                     if nc.partition_id_tensor else None)
        in_names, out_names, out_avals = [], [], []
        for alloc in nc.m.functions[0].allocations:
            if not isinstance(alloc, _mb.MemoryLocationSet):
                continue
            name = alloc.memorylocations[0].name
            if alloc.kind == "ExternalInput":
                if name != part_name:
                    in_names.append(name)
            elif alloc.kind == "ExternalOutput":
                out_names.append(name)
                out_avals.append(jax.core.ShapedArray(
                    tuple(alloc.tensor_shape), _mb.dt.np(alloc.dtype)))
        self.in_names, self.out_names, self.out_avals = in_names, out_names, out_avals
        n_params, n_outs = len(in_names), len(out_names)
        bind_names = tuple(in_names + out_names
                           + ([part_name] if part_name else []))

        def _body(*args):
            operands = list(args)
            if part_name:
                operands.append(partition_id_tensor())
            return tuple(_bass_exec_p.bind(
                *operands, out_avals=tuple(out_avals), in_names=bind_names,
                out_names=tuple(out_names), lowering_input_output_aliases=(),
                sim_require_finite=True, sim_require_nnan=True, nc=nc))

        devices = jax.devices()[:NCORES]
        self.mesh = Mesh(np.asarray(devices), ("core",))
        in_specs = (PartitionSpec("core"),) * (n_params + n_outs)
        out_specs = (PartitionSpec("core"),) * n_outs
        self.sharded = jax.jit(
            shard_map(_body, mesh=self.mesh, in_specs=in_specs,
                      out_specs=out_specs, check_rep=False),
            donate_argnums=tuple(range(n_params, n_params + n_outs)),
            keep_unused=True)
        self.sharding = jax.sharding.NamedSharding(
            self.mesh, PartitionSpec("core"))

    def put_inputs(self, in_maps):
        """Concatenate per-core inputs on axis 0 and place on devices."""
        concat = [np.concatenate([np.asarray(m[n]) for m in in_maps], axis=0)
                  for n in self.in_names]
        return [self.jax.device_put(a, self.sharding) for a in concat]

    def zeros(self):
        return [self.jax.device_put(
                    np.zeros((NCORES * av.shape[0], *av.shape[1:]), av.dtype),
                    self.sharding)
                for av in self.out_avals]

    def run(self, dev_inputs, dev_zeros=None):
        if dev_zeros is None:
            dev_zeros = self.zeros()
        outs = self.sharded(*dev_inputs, *dev_zeros)
        self.jax.block_until_ready(outs)
        return outs

    def run_np(self, dev_inputs):
        outs = self.run(dev_inputs)
        res = {}
        for i, name in enumerate(self.out_names):
            a = np.asarray(outs[i])
            res[name] = a.reshape(NCORES, *self.out_avals[i].shape)
        return res


_RUNNERS = {}


def _get_runner(reps=1):
    if reps not in _RUNNERS:
        _RUNNERS[reps] = _Runner(_get_program(reps))
    return _RUNNERS[reps]


def kernel(x, ln_w, ln_b, qkv_w, qkv_b, proj_w, proj_b,
           attn_biases, bias_idxs):
    x, ln_w, ln_b, qkv_w, qkv_b, proj_w, proj_b, attn_biases, bias_idxs = (
        np.asarray(a) for a in (x, ln_w, ln_b, qkv_w, qkv_b, proj_w, proj_b,
                                attn_biases, bias_idxs))
    runner = _get_runner()
    shared = _prep_inputs(x, ln_w, ln_b, qkv_w, qkv_b, proj_w, proj_b,
                          attn_biases, bias_idxs)
    in_maps = _make_in_maps(np.asarray(x), shared)
    dev = runner.put_inputs(in_maps)
    out = runner.run_np(dev)["out"]          # [NCORES, BL, N, DIM]
    return out.reshape(B, N, DIM).astype(np.float32)


# revision 14
# speedup vs baseline: 1.2835x; 1.2835x over previous
"""Trainium2 Bass kernel for nn_Attention_62672162783289.

Dense transformer attention block: LayerNorm -> fused QKV -> per-head scaled
dot-product attention with gathered relative-position bias -> output proj.

Sharding: data-parallel over batch B=16 across 8 NeuronCores (2 batches/core).
No collectives needed; outputs are concatenated on the host.

Device-side design (per core, all matmuls bf16 with fp32 PSUM accumulation):
  - The kernel is PE-bound: QKV (221k col-cycles) + scores/AV (393k) + proj
    (74k) is the irreducible bf16 matmul work. Everything else is arranged to
    stay off the PE and below the ScalarE exp stream (which sets the phase-2
    step period at ~16.6us: 16 x [128,1024] exp activations per head-pair).
  - LayerNorm stats run on VectorE (bn_stats/bn_aggr over a second token-major
    copy of x), not as ones-matmuls: PE does no stats work, and ScalarE does
    no Square pass. alpha/beta rows bounce through DRAM for the partition
    broadcast used by the xhat multiply (DVE).
  - rstd = exp(-0.5*ln(var+eps)) on ScalarE keeps every activation in the
    natural_log_exp_and_others table set: zero ACT table swaps in the kernel.
  - Q,K are produced feature-major in head-pair tiles (q_2p|q_2p+1 /
    k_2p|k_2p+1 on partition halves); scores are computed transposed S^T[m,n]
    so softmax normalization is a column sum obtained for free from an extra
    ones-column in the AV lhsT.
  - The position bias is applied per (mt,nb) tile according to PLAN: 'E'
    multiplies a precomputed exp(bias) into the exp'd scores on VectorE; 'I'
    adds the pre-scaled bias with identity matmuls on the PE. The mix
    balances DVE/PE load under the ScalarE exp period. (PSUM seeding via
    GpSimd or DMA is rejected by walrus — only PE/ACT/DVE reach PSUM.)
  - V is produced token-major [m, (h,65)] with an interleaved ones column so it
    can be used directly as the AV stationary operand; the V bias commutes
    through the softmax average and is folded into the proj bias on the host.
  - AV output arrives transposed [(d|ones), n]; after dividing by the ones-row
    (denominator, bcast via DRAM bounce), it is exactly the proj contraction
    layout.
"""
import os
import numpy as np
import ml_dtypes

import concourse.bass as bass
import concourse.tile as tile
from concourse import bacc, mybir

bf16 = mybir.dt.bfloat16
f32 = mybir.dt.float32
FP = mybir.ActivationFunctionType
ALU = mybir.AluOpType

B, RES, DIM, H, KD = 16, 32, 768, 12, 64
N = RES * RES            # 1024 tokens
DH = KD * H              # 768
NCORES = 8
BL = B // NCORES         # 2 batches per core
SCALE = KD ** -0.5
LN_EPS = 1e-5
CHUNKS = DIM // 128      # 6 contraction chunks
PAIRS = H // 2           # 6 head pairs
MT = N // 128            # 8 m-tiles
TT = N // 128            # 8 token tiles

# Bias application plan per (mt, nb) tile (k = 2*mt+nb), repeated each step.
# 'E' = exp(bias) multiply on DVE, 'I' = PE ident matmul. (PSUM seeding via
# GpSimd/DMA is rejected by walrus: only PE/ACT/DVE can touch PSUM.)
PLAN = os.environ.get("BIAS_PLAN", "EEEEEIEEEEIEEEEI")
assert len(PLAN) == 16 and set(PLAN) <= set("EI")


def _bcast_ap(dram_ap, nparts):
    """Partition-step-0 broadcast AP over a DRAM row region."""
    return bass.AP(tensor=dram_ap.tensor, offset=dram_ap.offset,
                   ap=[[0, nparts]] + dram_ap.ap[1:])


def build_program(reps=1):
    nc = bacc.Bacc("TRN2", target_bir_lowering=False, debug=False,
                   num_devices=NCORES)

    x_d = nc.dram_tensor("x", [BL, DIM, N], bf16, kind="ExternalInput").ap()
    xt_d = nc.dram_tensor("xt", [BL, TT, 128, DIM], bf16,
                          kind="ExternalInput").ap()
    wqk_d = nc.dram_tensor("wqk", [DIM, 2 * DH], bf16, kind="ExternalInput").ap()
    qkb_d = nc.dram_tensor("qkb", [128, H], f32, kind="ExternalInput").ap()
    wv_d = nc.dram_tensor("wv", [DIM, DH], bf16, kind="ExternalInput").ap()
    pw_d = nc.dram_tensor("pw", [DH, DIM], bf16, kind="ExternalInput").ap()
    pb_d = nc.dram_tensor("pb", [1, DIM], f32, kind="ExternalInput").ap()
    eb_d = nc.dram_tensor("eb", [H, MT, 128, N], bf16, kind="ExternalInput").ap()
    sb_d = nc.dram_tensor("sb", [H, MT, 128, N], bf16, kind="ExternalInput").ap()
    id_d = nc.dram_tensor("ident", [128, 128], bf16, kind="ExternalInput").ap()
    out_d = nc.dram_tensor("out", [BL, N, DIM], f32, kind="ExternalOutput").ap()

    den_scr = nc.dram_tensor("den_scr", [PAIRS * BL, 4 * 512], f32).ap()
    ab_scr = nc.dram_tensor("ab_scr", [BL, N], bf16).ap()
    bb_scr = nc.dram_tensor("bb_scr", [BL, N], bf16).ap()

    with tile.TileContext(nc) as tc:
        with (
            tc.tile_pool(name="persist", bufs=1) as persist,
            tc.tile_pool(name="qkvout", bufs=1) as qkvout,
        ):
            # ---- persistent weights / constants (loads are emitted in
            # _emit_body so their descriptor-gen lands after the
            # latency-critical phase-1 loads in the queue order)
            pw_sb = [persist.tile([128, DIM], bf16, tag=f"pw{p}",
                                  name=f"pw{p}") for p in range(PAIRS)]
            projbB = persist.tile([128, DIM], f32, tag="projbB")
            ident = persist.tile([128, 128], bf16, tag="ident")

            for rep in range(reps):
                _emit_body(nc, tc, persist, qkvout, rep,
                           x_d, xt_d, wqk_d, qkb_d, wv_d, eb_d, sb_d,
                           out_d, den_scr, ab_scr, bb_scr, pw_sb, projbB,
                           ident, pw_d, pb_d, id_d)

    nc.compile()
    return nc


def _emit_body(nc, tc, persist, qkvout, rep,
               x_d, xt_d, wqk_d, qkb_d, wv_d, eb_d, sb_d, out_d,
               den_scr, ab_scr, bb_scr, pw_sb, projbB, ident, pw_d, pb_d,
               id_d):
    # tags are shared across reps: WAR deps serialize reps (correct; for timing)
    r = ""
    qk_sb = [[None] * H for _ in range(BL)]  # 12 f-tiles: even=QPAIR, odd=KPAIR
    v_sb = [[None] * TT for _ in range(BL)]  # [b][tt] -> [128, H*65] bf16
    projIn = [[None] * PAIRS for _ in range(BL)]

    # ======== Phase 1: LayerNorm + QKV ========
    # Stats on DVE (bn_stats over token-major x), QKV on PE. Loads are spread
    # over the sync/scalar/vector/gpsimd DMA queues so x, xt and wqk stream in
    # parallel and the first QKV matmul can start ~8us in.
    with (
        tc.tile_pool(name="wqkp", bufs=1) as wqkp,
        tc.tile_pool(name="xp", bufs=12) as xp,
        tc.tile_pool(name="xtp", bufs=8) as xtp,
        tc.tile_pool(name="bnp", bufs=2) as bnp,
        tc.tile_pool(name="rowp", bufs=2) as rowp,
        tc.tile_pool(name="lnbc", bufs=2) as lnbc,
        tc.tile_pool(name="xhp", bufs=12) as xhp,
        tc.tile_pool(name="tmpp", bufs=2) as tmpp,
        tc.tile_pool(name="qkps", bufs=2, space="PSUM") as qkps,
        tc.tile_pool(name="vps", bufs=2, space="PSUM") as vps,
    ):
        # Load-queue assignment is startup-latency critical: ScalarE's queue
        # carries only the xt tiles (its descriptor-gen finishes before the
        # first rowmath activation), the sync queue carries xc(b0)+wqk+ident
        # first so the first QKV matmul can start ~15us in, and everything
        # heavier is emitted after the stats stages so its descriptor-gen
        # queues behind the critical path.
        xtc = [[None] * TT for _ in range(BL)]
        for b in range(BL):
            for t in range(TT):
                tl = xtp.tile([128, DIM], bf16, name="xtc", tag="xtc")
                nc.scalar.dma_start(tl[:], xt_d[b, t])
                xtc[b][t] = tl
        xc = [[None] * CHUNKS for _ in range(BL)]
        for c in range(CHUNKS):
            t = xp.tile([128, N], bf16, name="xc", tag="xc")
            nc.sync.dma_start(t[:], x_d[0, c * 128:(c + 1) * 128, :])
            xc[0][c] = t
        wqk_sb, wv_sb = [], []
        for c in range(CHUNKS):
            t = wqkp.tile([128, 2 * DH], bf16, tag=f"wqk{c}")
            nc.sync.dma_start(t[:], wqk_d[c * 128:(c + 1) * 128, :])
            wqk_sb.append(t)
        nc.sync.dma_start(ident[:], id_d[:])
        eps_t = wqkp.tile([128, 1], f32, tag="eps")
        nc.vector.memset(eps_t[:], LN_EPS)
        # dummy activation: pull the reciprocal_sqrt table load off the
        # critical path (runs during the initial DMA wait)
        dum = wqkp.tile([128, 1], f32, tag="dum")
        nc.scalar.activation(dum[:], eps_t[:], FP.Sqrt)

        def emit_late_loads():
            qkbT = wqkp.tile([128, H], f32, tag="qkbT")
            nc.sync.dma_start(qkbT[:], qkb_d[:])
            for c in range(CHUNKS):
                t = wqkp.tile([128, DH], bf16, tag=f"wv{c}")
                nc.sync.dma_start(t[:], wv_d[c * 128:(c + 1) * 128, :])
                wv_sb.append(t)
            for c in range(CHUNKS):
                t = xp.tile([128, N], bf16, name="xc", tag="xc")
                nc.sync.dma_start(t[:], x_d[1, c * 128:(c + 1) * 128, :])
                xc[1][c] = t
            for p in range(PAIRS):
                nc.sync.dma_start(pw_sb[p][:], pw_d[p * 128:(p + 1) * 128, :])
            nc.gpsimd.dma_start(projbB[:], _bcast_ap(pb_d[0:1, :], 128))
            return qkbT

        bcasts = {}

        def stage_stats(b, h):
            """Stats for token tiles 4h..4h+3: bn_stats/bn_aggr on DVE,
            alpha = rsqrt(var+eps) on ACT, beta = -mu*alpha on DVE. The
            per-token alpha|beta columns are PE-transposed so both DRAM-bounce
            DMAs move contiguous 256B runs (a [128,1]-column DMA would be 128
            2-byte descriptors), then partition-broadcast back via gpsimd."""
            st4 = rowp.tile([128, 4, 2], f32, name="st4", tag="st4")
            for i in range(4):
                t = 4 * h + i
                bn6 = bnp.tile([128, 2, 6], f32, name="bn6", tag="bn6")
                nc.vector.bn_stats(bn6[:, 0, :], xtc[b][t][:, 0:384])
                nc.vector.bn_stats(bn6[:, 1, :], xtc[b][t][:, 384:768])
                nc.vector.bn_aggr(st4[:, i, :], bn6[:])
            s4 = rowp.tile([128, 4], f32, name="s4", tag="s4")
            nc.scalar.activation(s4[:], st4[:, :, 1], FP.Sqrt, bias=eps_t[:])
            a4 = rowp.tile([128, 8], bf16, name="a4", tag="a4")
            with nc.allow_low_precision(reason="alpha rows are bf16 by design"):
                nc.vector.reciprocal(a4[:, 0:4], s4[:])
            nc.vector.scalar_tensor_tensor(a4[:, 4:8], st4[:, :, 0], -1.0,
                                           a4[:, 0:4], ALU.mult, ALU.mult)
            tp = qkps.tile([8, 128], bf16, name="tp", tag="tp")
            nc.tensor.transpose(tp[:], a4[:], ident[:])
            abT = rowp.tile([8, 128], bf16, name="abT", tag="abT")
            nc.vector.tensor_copy(abT[:], tp[:])
            a_dst = bass.AP(tensor=ab_scr.tensor,
                            offset=ab_scr.offset + b * N + h * 512,
                            ap=[[128, 4], [1, 128]])
            b_dst = bass.AP(tensor=bb_scr.tensor,
                            offset=bb_scr.offset + b * N + h * 512,
                            ap=[[128, 4], [1, 128]])
            nc.sync.dma_start(a_dst, abT[0:4, :])
            nc.sync.dma_start(b_dst, abT[4:8, :])
            ts = slice(h * 512, (h + 1) * 512)
            alphaB = lnbc.tile([128, 512], bf16, tag="alphaB")
            nc.gpsimd.dma_start(alphaB[:], _bcast_ap(ab_scr[b:b + 1, ts], 128))
            betaB = lnbc.tile([128, 512], bf16, tag="betaB")
            nc.gpsimd.dma_start(betaB[:], _bcast_ap(bb_scr[b:b + 1, ts], 128))
            bcasts[(b, h)] = (alphaB, betaB)

        xh = [[None] * CHUNKS for _ in range(BL)]

        def stage_c(b, tb):  # xhat = x*alpha + beta (bf16), one t-half
            alphaB, betaB = bcasts[(b, tb)]
            ts = slice(tb * 512, (tb + 1) * 512)
            for c in range(CHUNKS):
                if xh[b][c] is None:
                    xh[b][c] = xhp.tile([128, N], bf16, name="xh", tag="xh")
                t0 = tmpp.tile([128, 512], bf16, name="t0", tag="t0")
                nc.vector.tensor_mul(t0[:], xc[b][c][:, ts], alphaB[:])
                nc.vector.tensor_add(xh[b][c][:, ts], t0[:], betaB[:])

        def stage_d_qk(b, tb):  # QK matmuls for one t-half
            for ft in range(H):
                if qk_sb[b][ft] is None:
                    qk_sb[b][ft] = qkvout.tile([128, N], bf16,
                                               tag=f"{r}qk{b}_{ft}", name="qt")
                qt = qk_sb[b][ft]
                ps = qkps.tile([128, 512], f32, name="qkp", tag="qkp")
                for c in range(CHUNKS):
                    nc.tensor.matmul(
                        ps[:], wqk_sb[c][:, ft * 128:(ft + 1) * 128],
                        xh[b][c][:, tb * 512:(tb + 1) * 512],
                        start=(c == 0), stop=(c == CHUNKS - 1))
                nc.scalar.activation(qt[:, tb * 512:(tb + 1) * 512], ps[:],
                                     FP.Identity, bias=qkbT[:, ft:ft + 1])

        def stage_d_v(b, tts):  # V matmuls for the given token tiles
            for tt in tts:
                ps = vps.tile([128, DH], f32, name="vp", tag="vp")
                for c in range(CHUNKS):
                    lhs = xh[b][c][:, tt * 128:(tt + 1) * 128]
                    nc.tensor.matmul(ps[:, 0:512], lhs, wv_sb[c][:, 0:512],
                                     start=(c == 0), stop=(c == CHUNKS - 1))
                    nc.tensor.matmul(ps[:, 512:DH], lhs, wv_sb[c][:, 512:DH],
                                     start=(c == 0), stop=(c == CHUNKS - 1))
                # V bias is folded into the proj bias on the host (the AV
                # normalize makes it a constant per-dh additive term), so the
                # evacuation is a plain copy: on ScalarE, which idles in
                # phase 1, keeping VectorE free for stats/xhat.
                vt = qkvout.tile([128, H * 65], bf16, tag=f"{r}v{b}_{tt}")
                v_sb[b][tt] = vt
                vv = vt[:].rearrange("p (h d) -> p h d", d=65)
                nc.scalar.activation(
                    vv[:, :, 0:64],
                    ps[:].rearrange("p (h d) -> p h d", d=64), FP.Identity)
                nc.vector.memset(vv[:, :, 64:65], 1.0)

        stage_stats(0, 0)
        stage_stats(0, 1)
        stage_c(0, 0)
        stage_c(0, 1)
        qkbT = emit_late_loads()
        stage_d_qk(0, 0)
        stage_stats(1, 0)            # DVE prepares b1 while PE runs QKV(b0)
        stage_stats(1, 1)
        # dummy exp: pull the exp table load into phase-1 ACT slack
        nc.scalar.activation(dum[:], eps_t[:], FP.Exp)
        stage_d_v(0, range(0, 4))
        stage_c(1, 0); stage_c(1, 1)
        stage_d_qk(0, 1); stage_d_v(0, range(4, TT))
        stage_d_qk(1, 0); stage_d_v(1, range(0, 4))
        stage_d_qk(1, 1); stage_d_v(1, range(4, TT))

    # ======== Phase 2: attention + projection, software-pipelined ========
    # Step i emits: scores+exp(+bias per PLAN) of i with the AV matmuls of i-1
    # interleaved per m-chunk, the deferred reciprocal+normalize of i-2
    # mid-step, and the AV-accumulator evacuation of i-1. proj(b0) is emitted
    # before the pipeline drain so the tensor engine has dense work while the
    # last step's exps stream through ScalarE.
    with (
        tc.tile_pool(name="ebp", bufs=20) as ebp,
        tc.tile_pool(name="ep", bufs=20) as ep,
        tc.tile_pool(name="denrp", bufs=1) as denrp,
        tc.tile_pool(name="recp", bufs=3) as recp,
        tc.tile_pool(name="tmpb", bufs=2) as tmpb,
        tc.tile_pool(name="outp", bufs=3) as outp,
        tc.tile_pool(name="sps", bufs=2, space="PSUM") as spsp,
        tc.tile_pool(name="avps", bufs=2, space="PSUM") as avpsp,
    ):
        steps = [(p, b) for p in range(PAIRS) for b in range(BL)]
        eb_cache = {}

        def load_bias(p):
            # tile k = (mt, nb): both heads' [128, 512] n-block side by side.
            # 'E' tiles hold exp(bias) (multiplied on VectorE); 'S'/'I' tiles
            # hold the pre-scaled additive bias.
            eb_sb = [None] * (2 * MT)
            for mt in range(MT):
                for nb in range(2):
                    k = 2 * mt + nb
                    t = ebp.tile([128, N], bf16, name="ebt", tag="ebt")
                    src_d = eb_d if PLAN[k] == 'E' else sb_d
                    base = src_d[2 * p, mt]
                    src = bass.AP(tensor=base.tensor,
                                  offset=base.offset + nb * 512,
                                  ap=[base.ap[0], [MT * 128 * N, 2], [1, 512]])
                    eng = nc.gpsimd if PLAN[k] == 'E' else nc.sync
                    eng.dma_start(
                        t[:].rearrange("p (h n) -> p h n", h=2), src)
                    eb_sb[k] = t
            return eb_sb

        def av_chunk(st, mt):
            p, b, E, avt = st["p"], st["b"], st["E"], st["avt"]
            for hh in range(2):
                h = 2 * p + hh
                lhsT = v_sb[b][mt][:, h * 65:(h + 1) * 65]
                for nb in range(2):
                    nc.tensor.matmul(
                        avt[hh][:, nb * 512:(nb + 1) * 512], lhsT,
                        E[2 * mt + nb][:, hh * 512:(hh + 1) * 512],
                        start=(mt == 0), stop=(mt == MT - 1))

        def evac_step(st):
            """Denominators out + unnormalized PSUM->SBUF evacuation, with the
            copies split across ScalarE/VectorE to balance their step load."""
            p, b, avt = st["p"], st["b"], st["avt"]
            denrow = denrp.tile([65, 4 * 512], f32, name="denrow", tag="denrow")
            # avt[hh] row 64 is [nb0|nb1] contiguous == den_scr quadrant order
            nc.vector.tensor_copy(denrow[64:65, 0:N], avt[0][64:65, :])
            nc.vector.tensor_copy(denrow[64:65, N:2 * N], avt[1][64:65, :])
            PI = qkvout.tile([128, N], bf16, tag=f"{r}qk{b}_{2 * p}")
            projIn[b][p] = PI
            tb_t = tmpb.tile([64, N], bf16, name="tbt", tag="tbt")
            nc.vector.tensor_copy(PI[0:64, :], avt[0][0:64, :])
            nc.vector.tensor_copy(tb_t[:], avt[1][0:64, :])
            nc.sync.dma_start(PI[64:128, :], tb_t[:])
            idx = p * BL + b
            nc.sync.dma_start(den_scr[idx:idx + 1, :], denrow[64:65, :])
            denB = recp.tile([128, N], f32, tag="recp", name="denB")
            for hh in range(2):
                for nb in range(2):
                    q = hh * 2 + nb
                    src = den_scr[idx:idx + 1, q * 512:(q + 1) * 512]
                    nc.gpsimd.dma_start(
                        denB[hh * 64:(hh + 1) * 64, nb * 512:(nb + 1) * 512],
                        _bcast_ap(src, 64))
            st["denB"] = denB
            st["PI"] = PI

        def norm_step(st):
            rB = recp.tile([128, N], f32, tag="recp", name="rB")
            nc.vector.reciprocal_approx_fast(rB[:], st["denB"][:])
            nc.vector.tensor_mul(st["PI"][:], st["PI"][:], rB[:])

        def emit_proj(b):
            for tt in range(TT):
                ps = spsp.tile([128, N], f32, name="sps", tag="sps")
                for p in range(PAIRS):
                    lhsT = projIn[b][p][:, tt * 128:(tt + 1) * 128]
                    nc.tensor.matmul(ps[:, 0:512], lhsT, pw_sb[p][:, 0:512],
                                     start=(p == 0), stop=(p == PAIRS - 1))
                    nc.tensor.matmul(ps[:, 512:DIM], lhsT, pw_sb[p][:, 512:DIM],
                                     start=(p == 0), stop=(p == PAIRS - 1))
                ot = outp.tile([128, DIM], f32, name="ot", tag="ot")
                nc.vector.tensor_add(ot[:], ps[:, 0:DIM], projbB[:])
                nc.scalar.dma_start(out_d[b, tt * 128:(tt + 1) * 128, :], ot[:])

        prev = None
        pend = None
        for p, b in steps:
            if b == 0:
                eb_cache[p] = load_bias(p)
            eb_sb = eb_cache[p]
            QP = qk_sb[b][2 * p]
            KP = qk_sb[b][2 * p + 1]
            E = [None] * (2 * MT)
            cur = {"p": p, "b": b, "E": E,
                   "avt": [avpsp.tile([65, N], f32, name="avt", tag="avt")
                           for _ in range(2)]}
            for mt in range(MT):
                # half-wide score tiles (both heads, one n-block) double-
                # buffered: the next tile's matmuls only wait on the exp two
                # tiles back, breaking the PE<->ScalarE alternation chain
                for nb in range(2):
                    k = 2 * mt + nb
                    v = PLAN[k]
                    sps = spsp.tile([128, N], f32, name="sps", tag="sps")
                    bt = eb_sb[k]
                    for hh in range(2):
                        sl = slice(hh * 64, (hh + 1) * 64)
                        lhsT = KP[sl, mt * 128:(mt + 1) * 128]
                        nc.tensor.matmul(
                            sps[:, hh * 512:(hh + 1) * 512], lhsT,
                            QP[sl, nb * 512:(nb + 1) * 512],
                            start=True, stop=(v != 'I'))
                    if v == 'I':
                        for hh in range(2):
                            nc.tensor.matmul(
                                sps[:, hh * 512:(hh + 1) * 512], ident[:],
                                bt[:, hh * 512:(hh + 1) * 512],
                                start=False, stop=True)
                    et = ep.tile([128, N], bf16, name="et", tag="et")
                    nc.scalar.activation(et[:], sps[:], FP.Exp, scale=SCALE)
                    if v == 'E':
                        nc.vector.tensor_mul(et[:], et[:], bt[:])
                    E[k] = et
                if prev is not None:
                    av_chunk(prev, mt)
                if mt == 3 and pend is not None:
                    norm_step(pend)
                    pend = None
            if prev is not None:
                evac_step(prev)
                pend = prev
            prev = cur
        # normalize the second-to-last step, then emit proj(b0): it fills the
        # PE while the final step's exps drain through ScalarE
        if pend is not None:
            norm_step(pend)
            pend = None
        emit_proj(0)
        # drain the pipeline
        for mt in range(MT):
            av_chunk(prev, mt)
        evac_step(prev)
        norm_step(prev)
        emit_proj(1)


# ---------------- host side ----------------

def _prep_inputs(x, ln_w, ln_b, qkv_w, qkv_b, proj_w, proj_b,
                 attn_biases, bias_idxs):
    """Fold LN affine into QKV weights; build device layouts (shared part)."""
    f64 = np.float64
    Wp = qkv_w.astype(f64) * ln_w.astype(f64)[None, :]       # [2304, 768]
    bp = qkv_b.astype(f64) + qkv_w.astype(f64) @ ln_b.astype(f64)

    def q_rows(h): return np.arange(h * 3 * KD, h * 3 * KD + KD)
    def k_rows(h): return np.arange(h * 3 * KD + KD, h * 3 * KD + 2 * KD)
    def v_rows(h): return np.arange(h * 3 * KD + 2 * KD, h * 3 * KD + 3 * KD)

    qk_order = []
    for p in range(PAIRS):
        qk_order += list(q_rows(2 * p)) + list(q_rows(2 * p + 1))
        qk_order += list(k_rows(2 * p)) + list(k_rows(2 * p + 1))
    qk_order = np.array(qk_order)
    v_order = np.concatenate([v_rows(h) for h in range(H)])

    wqk = np.ascontiguousarray(Wp[qk_order].T).astype(ml_dtypes.bfloat16)
    qkb = np.ascontiguousarray(
        bp[qk_order].astype(np.float32).reshape(H, 128).T)  # [128, H]
    wv = np.ascontiguousarray(Wp[v_order].T).astype(ml_dtypes.bfloat16)
    pw = np.ascontiguousarray(proj_w.T).astype(ml_dtypes.bfloat16)
    # V bias (LN-folded qkv bias for the v rows) commutes through the
    # softmax average, so it folds exactly into the proj bias.
    bv = bp[v_order]                                         # [DH] natural dh
    pb = (proj_b.astype(f64) + proj_w.astype(f64) @ bv).astype(
        np.float32).reshape(1, DIM)

    biasT = attn_biases.astype(f64)[:, np.asarray(bias_idxs)].transpose(0, 2, 1)
    eb = np.ascontiguousarray(np.exp(biasT)).astype(
        ml_dtypes.bfloat16).reshape(H, MT, 128, N)
    sb = np.ascontiguousarray(biasT / SCALE).astype(
        ml_dtypes.bfloat16).reshape(H, MT, 128, N)
    ident = np.eye(128, dtype=ml_dtypes.bfloat16)
    return dict(wqk=wqk, qkb=qkb, wv=wv, pw=pw, pb=pb, eb=eb, sb=sb,
                ident=ident)


def _make_in_maps(x, shared):
    xr = x.reshape(NCORES, BL, N, DIM)
    xft = np.ascontiguousarray(xr.transpose(0, 1, 3, 2)).astype(
        ml_dtypes.bfloat16)
    xtk = np.ascontiguousarray(xr.reshape(NCORES, BL, TT, 128, DIM)).astype(
        ml_dtypes.bfloat16)
    return [dict(x=xft[i], xt=xtk[i], **shared) for i in range(NCORES)]


_PROG = {}


def _get_program(reps=1):
    if reps not in _PROG:
        _PROG[reps] = build_program(reps)
    return _PROG[reps]


class _Runner:
    """Persistent jitted SPMD executor (mirrors bass2jax.run_bass_via_pjrt's
    multi-core branch, but the jitted callable is cached across calls)."""

    def __init__(self, nc):
        import jax
        from jax.experimental.shard_map import shard_map
        from jax.sharding import Mesh, PartitionSpec
        from concourse import mybir as _mb
        from concourse.bass2jax import _bass_exec_p, install_neuronx_cc_hook

        install_neuronx_cc_hook()
        self.jax = jax
        from concourse.bass2jax import partition_id_tensor
        part_name = (nc.partition_id_tensor.name
                     if nc.partition_id_tensor else None)
        in_names, out_names, out_avals = [], [], []
        for alloc in nc.m.functions[0].allocations:
            if not isinstance(alloc, _mb.MemoryLocationSet):
                continue
            name = alloc.memorylocations[0].name
            if alloc.kind == "ExternalInput":
                if name != part_name:
                    in_names.append(name)
            elif alloc.kind == "ExternalOutput":
                out_names.append(name)
                out_avals.append(jax.core.ShapedArray(
                    tuple(alloc.tensor_shape), _mb.dt.np(alloc.dtype)))
        self.in_names, self.out_names, self.out_avals = in_names, out_names, out_avals
        n_params, n_outs = len(in_names), len(out_names)
        bind_names = tuple(in_names + out_names
                           + ([part_name] if part_name else []))

        def _body(*args):
            operands = list(args)
            if part_name:
                operands.append(partition_id_tensor())
            return tuple(_bass_exec_p.bind(
                *operands, out_avals=tuple(out_avals), in_names=bind_names,
                out_names=tuple(out_names), lowering_input_output_aliases=(),
                sim_require_finite=True, sim_require_nnan=True, nc=nc))

        devices = jax.devices()[:NCORES]
        self.mesh = Mesh(np.asarray(devices), ("core",))
        in_specs = (PartitionSpec("core"),) * (n_params + n_outs)
        out_specs = (PartitionSpec("core"),) * n_outs
        self.sharded = jax.jit(
            shard_map(_body, mesh=self.mesh, in_specs=in_specs,
                      out_specs=out_specs, check_rep=False),
            donate_argnums=tuple(range(n_params, n_params + n_outs)),
            keep_unused=True)
        self.sharding = jax.sharding.NamedSharding(
            self.mesh, PartitionSpec("core"))

    def put_inputs(self, in_maps):
        """Concatenate per-core inputs on axis 0 and place on devices."""
        concat = [np.concatenate([np.asarray(m[n]) for m in in_maps], axis=0)
                  for n in self.in_names]
        return [self.jax.device_put(a, self.sharding) for a in concat]

    def zeros(self):
        return [self.jax.device_put(
                    np.zeros((NCORES * av.shape[0], *av.shape[1:]), av.dtype),
                    self.sharding)
                for av in self.out_avals]

    def run(self, dev_inputs, dev_zeros=None):
        if dev_zeros is None:
            dev_zeros = self.zeros()
        outs = self.sharded(*dev_inputs, *dev_zeros)
        self.jax.block_until_ready(outs)
        return outs

    def run_np(self, dev_inputs):
        outs = self.run(dev_inputs)
        res = {}
        for i, name in enumerate(self.out_names):
            a = np.asarray(outs[i])
            res[name] = a.reshape(NCORES, *self.out_avals[i].shape)
        return res


_RUNNERS = {}


def _get_runner(reps=1):
    if reps not in _RUNNERS:
        _RUNNERS[reps] = _Runner(_get_program(reps))
    return _RUNNERS[reps]


def kernel(x, ln_w, ln_b, qkv_w, qkv_b, proj_w, proj_b,
           attn_biases, bias_idxs):
    x, ln_w, ln_b, qkv_w, qkv_b, proj_w, proj_b, attn_biases, bias_idxs = (
        np.asarray(a) for a in (x, ln_w, ln_b, qkv_w, qkv_b, proj_w, proj_b,
                                attn_biases, bias_idxs))
    runner = _get_runner()
    shared = _prep_inputs(x, ln_w, ln_b, qkv_w, qkv_b, proj_w, proj_b,
                          attn_biases, bias_idxs)
    in_maps = _make_in_maps(np.asarray(x), shared)
    dev = runner.put_inputs(in_maps)
    out = runner.run_np(dev)["out"]          # [NCORES, BL, N, DIM]
    return out.reshape(B, N, DIM).astype(np.float32)
